# revision 26
# baseline (speedup 1.0000x reference)
"""GNN encoder (Linear+ReLU -> mean-aggregation SAGEConv) on 8 TRN2 NeuronCores.

Self-contained: hardcodes problem shapes (N=100000, XD=512, HID=64, E=1e6).

Strategy (v3):
  - Nodes sharded across 8 cores (12500 each, padded to 12544 = 98 tiles).
  - Phase 1 per core: hT = relu(W1 @ xT + b1) via PE; x fed host-pretransposed
    in PE-ready [128, 4, 512] group layout. hT kept in SBUF (bf16) for the
    combine's root term.
  - h rows are PE-transposed, cast to fp8e4m3 (64B payload in a 256B-stride
    row, required by the SWDGE gather), and stored to ag_in with a global
    partition-major swizzle (node i -> row (i%128)*98 + i//128, so each
    group store is one 128-descriptor DMA).
  - One AllGather -> full 100352-row fp8 table (4 int16-addressable banks of
    25088 rows). A local pseudo-bank was tried and removed: the AllGather's
    own transfers occupy the DMA engines during that window, so overlapping
    local gathers is zero-sum (total DMA work is conserved). Splitting the
    AllGather (per-bank or per-half) also loses: each collective pays ~25us
    of inter-core sync and the later halves contend with phase-2 gathers.
  - Edges partitioned by destination node; per core grouped by (dst tile,
    src bank) in (28-tile block, bank, tile) order; chunks of 128 with a
    shared max-over-cores schedule (SPMD: one program for all 8 cores);
    instrs batch <=6 same-bank chunks (SWDGE ring holds 1024 descriptors;
    6-chunk instrs measured fastest).
  - dma_gather (4 SWDGE rings, ~25-32ns/row: HBM random-access bound)
    fetches fp8 h[src] rows; GpSimd issue time is backpressure-dominated.
  - One-hot B matrices generated ON DEVICE: one DVE tensor_tensor is_equal
    per gather instr (iota row vs per-chunk dstloc, stride-0 broadcast APs),
    fp8 0/1 output. This replaces the ~20MB host-precomputed B stream the
    previous version fetched from HBM.
  - Per chunk: PE matmul lhsT=msg[128,64] fp8 x rhs=B[128,128] accumulates
    sums into per-quad PSUM [64,512] (whole-quad start/stop flags since
    start zeroes the full bank; 7 rotating tiles per 28-tile block).
  - Combine per quad: meanT = sums * minv (full 1/deg in bf16), then
    cps = meanT.T @ WlT + hT.T @ WrT + bl; batched stores; bf16 out, host
    upcasts.  Rel err ~0.0174 (fp8 message noise; gate is 2e-2).
"""

import numpy as np
import ml_dtypes

N_NODES = 100000
XD = 512
HID = 64
N_CORES = 8
SH = N_NODES // N_CORES          # 12500
P = 128
T_TILES = 98                     # ceil(12500/128)
SHP = T_TILES * P                # 12544
NTAB = SHP * N_CORES             # 100352
N_BANKS = 4
BANK = NTAB // N_BANKS           # 25088 (int16-addressable)
BLOCK_TILES = 28                 # tiles per psum block (7 quads)
MAX_CHUNKS_PER_INSTR = 6         # 1024 descriptors (runtime SWDGE ring cap)
SCRATCH = 16384
N_QUADS = (T_TILES + 3) // 4     # 25
GROUPS = [(g * 512, min(512, SHP - g * 512)) for g in range((SHP + 511) // 512)]
# ag_in row order is swizzled so the single phase-1 store is partition-major:
# local node i -> row (i%128)*T_TILES + i//128
_i = np.arange(SHP)
ROW_SWIZ = (_i % P) * T_TILES + _i // P

BLOCKS = [list(range(b0, min(b0 + BLOCK_TILES, T_TILES)))
          for b0 in range(0, T_TILES, BLOCK_TILES)]

TRACE = False          # set True (e.g. from test.py) to profile
LAST_EXEC_NS = None    # filled when TRACE
LAST_RES = None


def _prep(edge_index):
    """Host-side sharding/scheduling. Returns shared schedule + per-core arrays.

    Groups: per dst tile, a LOCAL group (src in own shard, gathered from ltab
    before the AllGather completes) ordered tile-major first, then remote
    groups (4 table banks) in (block, bank, tile) order. Chunks of 128 edges;
    instructions batch <=8 consecutive same-bank chunks.
    """
    src = np.asarray(edge_index[0], dtype=np.int64)
    dst = np.asarray(edge_index[1], dtype=np.int64)
    LB = N_BANKS  # local pseudo-bank

    group_list = [(t, LB) for t in range(T_TILES)]
    for tiles in BLOCKS:
        for b in range(N_BANKS):
            for t in tiles:
                group_list.append((t, b))
    G = len(group_list)
    gid_of = {tb: i for i, tb in enumerate(group_list)}
    gid_lut = np.zeros((T_TILES, N_BANKS + 1), dtype=np.int64)
    for (t, b), i in gid_of.items():
        gid_lut[t, b] = i

    per_core = []
    counts_all = np.zeros((N_CORES, G), dtype=np.int64)
    for c in range(N_CORES):
        sel = (dst >= c * SH) & (dst < (c + 1) * SH)
        e_src = src[sel]
        e_ld = (dst[sel] - c * SH).astype(np.int64)
        deg = np.bincount(e_ld, minlength=SHP)
        minv = (1.0 / np.maximum(deg, 1)).astype(np.float32)
        # local bank disabled: the AG window is occupied by collective traffic
        tid = (e_src // SH) * SHP + ROW_SWIZ[e_src % SH]
        bank = tid // BANK
        blocal = (tid % BANK).astype(np.int64)
        tt = e_ld // P
        gid = gid_lut[tt, bank]
        order = np.argsort(gid * (BANK + 1) + blocal, kind="stable")
        per_core.append({
            "blocal": blocal[order].astype(np.int16),
            "dstloc": (e_ld[order] % P).astype(np.float32),
            "minv_row": minv,
        })
        counts_all[c] = np.bincount(gid, minlength=G)

    q_g = -(-counts_all.max(axis=0) // P)   # chunks per group (shared)

    sched_t = []
    sched_b = []
    for gi, (t, b) in enumerate(group_list):
        for _ in range(q_g[gi]):
            sched_t.append(t)
            sched_b.append(b)
    sched_t = np.array(sched_t, dtype=np.int64)
    sched_b = np.array(sched_b, dtype=np.int64)
    nch = len(sched_t)
    n_local = int((sched_b == LB).sum())

    # instruction list: batch consecutive same-bank chunks (within block for
    # remote; local chunks are all one pseudo-bank)
    instrs = []
    i = 0
    while i < nch:
        j = i
        while (j < nch and j - i < MAX_CHUNKS_PER_INSTR
               and sched_b[j] == sched_b[i]
               and (j < n_local) == (i < n_local)
               and (i < n_local
                    or sched_t[j] // BLOCK_TILES == sched_t[i] // BLOCK_TILES)):
            j += 1
        instrs.append((i, j - i, int(sched_b[i])))
        i = j

    lfirst = np.full(N_QUADS, -1, dtype=np.int64)
    llast = np.full(N_QUADS, -1, dtype=np.int64)
    rfirst = np.full(N_QUADS, -1, dtype=np.int64)
    rlast = np.full(N_QUADS, -1, dtype=np.int64)
    for ci in range(nch):
        q = sched_t[ci] // 4
        if ci < n_local:
            if lfirst[q] < 0:
                lfirst[q] = ci
            llast[q] = ci
        else:
            if rfirst[q] < 0:
                rfirst[q] = ci
            rlast[q] = ci

    # chunk slot offset within its group
    grp_seen = {}
    chunk_q = np.zeros(nch, dtype=np.int64)
    for ci in range(nch):
        k = (int(sched_t[ci]), int(sched_b[ci]))
        chunk_q[ci] = grp_seen.get(k, 0)
        grp_seen[k] = chunk_q[ci] + 1

    core_arrays = []
    for c in range(N_CORES):
        pc = per_core[c]
        cnts = counts_all[c]
        starts = np.zeros(G + 1, dtype=np.int64)
        np.cumsum(cnts, out=starts[1:])
        gidx = np.zeros((nch, P), dtype=np.int16)
        dstloc = np.full((nch, P), 255.0, dtype=np.float32)
        for ci in range(nch):
            t, b, qq = int(sched_t[ci]), int(sched_b[ci]), int(chunk_q[ci])
            g = gid_of[(t, b)]
            s0 = starts[g] + qq * P
            n = min(P, starts[g + 1] - s0)
            if n <= 0:
                continue
            sl = slice(s0, s0 + n)
            gidx[ci, :n] = pc["blocal"][sl]
            dstloc[ci, :n] = pc["dstloc"][sl]
        idx16 = gidx.reshape(nch, 8, 16).transpose(2, 0, 1).reshape(16, nch * 8)
        idx128 = np.tile(idx16, (8, 1))
        core_arrays.append({
            "gidx": np.ascontiguousarray(idx128),
            "dstloc": np.ascontiguousarray(dstloc.T),   # [128, nch]
            "minv": np.ascontiguousarray(
                np.broadcast_to(pc["minv_row"][None, :], (HID, SHP))
            ).astype(ml_dtypes.bfloat16),
        })

    # DoubleRow pairing: per instr, (k, n_k) runs of 1-2 same-tile chunks
    pair_runs = []
    for (c0, nch_i, b) in instrs:
        runs = []
        k = 0
        while k < nch_i:
            if (k + 1 < nch_i and sched_t[c0 + k] == sched_t[c0 + k + 1]):
                runs.append((k, 2))
                k += 2
            else:
                runs.append((k, 1))
                k += 1
        pair_runs.append(runs)

    meta = {
        "nch": nch,
        "n_local": n_local,
        "pair_runs": pair_runs,
        "instrs": instrs,
        "sched_t": sched_t,
        "lfirst": lfirst, "llast": llast,
        "rfirst": rfirst, "rlast": rlast,
        "has_chunks": np.array([
            counts_all.max(axis=0)[
                [gid_of[(t, b)] for b in range(N_BANKS + 1)]
            ].sum() > 0 for t in range(T_TILES)
        ]),
    }
    return meta, core_arrays


_GATHER_PATCHED = False


def _relax_gather_elem_assert():
    """dma_gather asserts elem_size_bytes % 256 == 0 (a transpose-mode
    restriction applied unconditionally). The non-transpose ucode handles
    128-byte payloads with a 256-byte row stride (verified on hardware)."""
    global _GATHER_PATCHED
    if _GATHER_PATCHED:
        return
    import inspect
    import re
    import concourse.bass as bassmod

    src = inspect.getsource(bassmod.BassGpSimd.dma_gather)
    src = src.replace(
        "elem_size_bytes > 0 and elem_size_bytes % 256 == 0",
        "elem_size_bytes > 0 and elem_size_bytes % 64 == 0",
    )
    src = re.sub(r"^    def ", "def ", src, count=1, flags=re.M)
    src = "\n".join(l[4:] if l.startswith("    ") else l for l in src.split("\n"))
    ns = dict(bassmod.__dict__)
    exec(compile(src, "patched_dma_gather", "exec"), ns)
    bassmod.BassGpSimd.dma_gather = ns["dma_gather"]
    _GATHER_PATCHED = True


def _build_program(meta):
    import concourse.bass as bass
    import concourse.bacc as bacc
    import concourse.mybir as mybir
    import concourse.tile as tile

    _relax_gather_elem_assert()

    nch = meta["nch"]
    gcols = nch * 8

    nc = bacc.Bacc("TRN2", target_bir_lowering=False, debug=False,
                   num_devices=N_CORES, num_swdge_queues=4,
                   dynamic_dma_scratch_size=SCRATCH)
    f32 = mybir.dt.float32
    bf16 = mybir.dt.bfloat16

    xg_in = nc.dram_tensor("xg", [P, 4 * SHP], bf16, kind="ExternalInput")
    w1t = nc.dram_tensor("w1t", [XD, HID], bf16, kind="ExternalInput")
    b1 = nc.dram_tensor("b1", [HID, 1], f32, kind="ExternalInput")
    wlt = nc.dram_tensor("wlt", [HID, HID], bf16, kind="ExternalInput")
    wrt = nc.dram_tensor("wrt", [HID, HID], bf16, kind="ExternalInput")
    blb = nc.dram_tensor("blb", [P, HID], f32, kind="ExternalInput")
    ident_in = nc.dram_tensor("ident", [HID, HID], bf16, kind="ExternalInput")
    gidx_in = nc.dram_tensor("gidx", [P, gcols], mybir.dt.int16, kind="ExternalInput")
    dstloc_in = nc.dram_tensor("dstloc", [P, nch], f32, kind="ExternalInput")
    iota_in = nc.dram_tensor("iota", [P, P], f32, kind="ExternalInput")
    minv_in = nc.dram_tensor("minv", [HID, SHP], bf16, kind="ExternalInput")

    NSTORE = sum(-(-len(t) // 4) for t in BLOCKS)
    out_d = nc.dram_tensor("out", [P, NSTORE * 4 * HID], bf16,
                           kind="ExternalOutput")

    fp8 = mybir.dt.float8e4
    ag_in = nc.dram_tensor("ag_in", [SHP, 4 * HID], fp8)
    ag_out = nc.dram_tensor("ag_out", [NTAB, 4 * HID], fp8, addr_space="Shared")

    with tile.TileContext(nc) as tc:
        with (
            tc.tile_pool(name="const", bufs=1) as cpool,
            tc.tile_pool(name="idx", bufs=1) as ipool,
            tc.tile_pool(name="hT", bufs=1) as hpool,
        ):
            w1t_sb = cpool.tile([P, 4, HID], bf16)
            nc.sync.dma_start(
                out=w1t_sb[:],
                in_=w1t.ap().rearrange("(k p) d -> p k d", p=P),
            )
            b1_sb = cpool.tile([HID, 1], f32)
            nc.sync.dma_start(out=b1_sb[:], in_=b1[:])
            wlt_sb = cpool.tile([HID, HID], bf16)
            nc.sync.dma_start(out=wlt_sb[:], in_=wlt[:])
            wrt_sb = cpool.tile([HID, HID], bf16)
            nc.sync.dma_start(out=wrt_sb[:], in_=wrt[:])
            blb_sb = cpool.tile([P, HID], f32)
            nc.sync.dma_start(out=blb_sb[:], in_=blb[:])
            ident_sb = cpool.tile([HID, HID], bf16)
            nc.sync.dma_start(out=ident_sb[:], in_=ident_in[:])
            gidx_sb = ipool.tile([P, gcols], mybir.dt.int16)
            nc.scalar.dma_start(out=gidx_sb[:], in_=gidx_in[:])
            dstloc_sb = ipool.tile([P, nch], f32)
            nc.scalar.dma_start(out=dstloc_sb[:], in_=dstloc_in[:])
            iota_sb = ipool.tile([P, P], f32)
            nc.scalar.dma_start(out=iota_sb[:], in_=iota_in[:])
            minv_sb = ipool.tile([HID, SHP], bf16)
            nc.scalar.dma_start(out=minv_sb[:], in_=minv_in[:])

            hT_sb = hpool.tile([HID, SHP], bf16)
            hrow_all = hpool.tile([P, T_TILES, 4 * HID], fp8)

            # ---------------- Phase 1: hT = relu(W1 @ xT + b1) ----------------
            with (
                tc.tile_pool(name="xg", bufs=6) as xpool,
                tc.tile_pool(name="p1ps", bufs=4, space="PSUM") as p1ps,
                tc.tile_pool(name="p1tr", bufs=4, space="PSUM") as p1tr,
            ):
                for gi, (g0, gw) in enumerate(GROUPS):
                    xt = xpool.tile([P, 4, 512], bf16, tag="xg")
                    xq = nc.scalar
                    xq.dma_start(
                        out=xt[:, :, :gw],
                        in_=xg_in.ap()[:, 4 * g0 : 4 * g0 + 4 * gw].rearrange(
                            "p (k j) -> p k j", k=4
                        ),
                    )
                    hps = p1ps.tile([HID, 512], f32, tag="hps", space="PSUM")
                    for k in range(4):
                        nc.tensor.matmul(
                            out=hps[:, :gw],
                            lhsT=w1t_sb[:, k, :],
                            rhs=xt[:, k, :gw],
                            start=(k == 0),
                            stop=(k == 3),
                        )
                    nc.scalar.activation(
                        out=hT_sb[:, g0 : g0 + gw], in_=hps[:, :gw],
                        func=mybir.ActivationFunctionType.Relu,
                        bias=b1_sb[:], scale=1.0,
                    )
                    ns = gw // P
                    for s in range(ns):
                        tp = p1tr.tile([P, HID], bf16, tag="tp", space="PSUM")
                        nc.tensor.transpose(
                            out=tp[:],
                            in_=hT_sb[:, g0 + s * P : g0 + (s + 1) * P],
                            identity=ident_sb[:],
                        )
                        nc.vector.tensor_copy(
                            out=hrow_all[:, g0 // P + s, :HID], in_=tp[:])
                    nc.sync.dma_start(
                        out=ag_in.ap().rearrange("(p t) d -> p t d", p=P)[
                            :, g0 // P : g0 // P + ns, :],
                        in_=hrow_all[:, g0 // P : g0 // P + ns, :],
                    )

            nc.gpsimd.collective_compute(
                "AllGather",
                mybir.AluOpType.bypass,
                replica_groups=[list(range(N_CORES))],
                ins=[ag_in.ap().opt()],
                outs=[ag_out.ap().opt()],
            )

            # ---------------- Phase 2: gather + aggregate + combine ----------
            LB = N_BANKS
            instrs = meta["instrs"]
            sched_t = meta["sched_t"]
            lfirst, llast = meta["lfirst"], meta["llast"]
            rfirst, rlast = meta["rfirst"], meta["rlast"]
            has_chunks = meta["has_chunks"]
            nch = meta["nch"]

            with (
                tc.tile_pool(name="msgbf", bufs=48) as mbfpool,
                tc.tile_pool(name="bmat", bufs=16) as bpool,
                tc.tile_pool(name="part", bufs=1) as ppool,
                tc.tile_pool(name="cps", bufs=1, space="PSUM") as cpspool,
                tc.tile_pool(name="comb", bufs=6) as combpool,
            ):
                cps_all = cpspool.tile([P, 2, HID], f32, tag="cps", space="PSUM")
                partials = {}
                n_comb = 0
                qn = 0

                def gather_and_btile(c0, nch_i, bank):
                    nonlocal qn
                    ni = nch_i * P
                    msgbf = mbfpool.tile([P, MAX_CHUNKS_PER_INSTR, HID], fp8,
                                         tag="msgbf")
                    src_ap = ag_out[bank * BANK : (bank + 1) * BANK, :HID]
                    nc.gpsimd.dma_gather(
                        msgbf[:, :nch_i, :],
                        src_ap,
                        gidx_sb[:, c0 * 8 : c0 * 8 + nch_i * 8],
                        ni, ni, HID,
                        elem_step=4 * HID,
                        queue_num=qn,
                    )
                    qn = (qn + 1) % 4
                    bt = bpool.tile([P, MAX_CHUNKS_PER_INSTR, P], fp8, tag="bt")
                    nc.vector.tensor_tensor(
                        out=bt[:, :nch_i, :],
                        in0=iota_sb[:].unsqueeze(1).broadcast_to([P, nch_i, P]),
                        in1=dstloc_sb[:, c0 : c0 + nch_i].unsqueeze(2)
                            .broadcast_to([P, nch_i, P]),
                        op=mybir.AluOpType.is_equal,
                    )
                    return msgbf, bt

                # ---- local phase: src in own shard, table = ltab ----
                n_local_instrs = 0
                with tc.tile_pool(name="lq", bufs=2, space="PSUM") as lqpool:
                    lq_tiles = {}
                    for ii, (c0, nch_i, bank) in enumerate(instrs):
                        if bank != LB:
                            break
                        n_local_instrs += 1
                        msgbf, btile = gather_and_btile(c0, nch_i, bank)
                        for k in range(nch_i):
                            ci = c0 + k
                            t = int(sched_t[ci])
                            q = t // 4
                            if q not in lq_tiles:
                                lq_tiles[q] = lqpool.tile(
                                    [HID, 512], f32, tag=f"lq{q % 2}",
                                    name=f"lq_{q}", space="PSUM"
                                )
                            lq = lq_tiles[q]
                            r = t - q * 4
                            nc.tensor.matmul(
                                out=lq[:, r * P : (r + 1) * P],
                                lhsT=msgbf[:, k, :],
                                rhs=btile[:, k, :],
                                start=(ci == lfirst[q]),
                                stop=(ci == llast[q]),
                            )
                            if ci == llast[q]:
                                par = ppool.tile([HID, 512], bf16,
                                                 tag=f"par{q}", name=f"par_{q}")
                                nc.vector.tensor_copy(out=par[:], in_=lq[:])
                                partials[q] = par

                # ---- remote phase ----
                with tc.tile_pool(name="agg", bufs=1, space="PSUM") as apool:
                    ptiles = {}

                    def ptile_of(q):
                        key = q % 7
                        if key not in ptiles or ptiles[key][1] != q:
                            ptiles[key] = (
                                apool.tile(
                                    [HID, 512], f32, tag=f"agg{key}",
                                    name=f"agg_{q}", space="PSUM"
                                ),
                                q,
                            )
                        return ptiles[key][0]

                    def emit_idadd(q, stop):
                        nc.tensor.matmul(
                            out=ptile_of(q)[:],
                            lhsT=ident_sb[:],
                            rhs=partials[q][:],
                            start=True, stop=stop,
                        )

                    def combine_quad(q):
                        tset = list(range(q * 4, min(q * 4 + 4, T_TILES)))
                        nonlocal n_comb
                        if rfirst[q] < 0 and q in partials:
                            emit_idadd(q, stop=True)
                        out_sb = combpool.tile([P, 4, HID], bf16, tag="outsb")
                        for si, t in enumerate(tset):
                            cps = cps_all[:, n_comb % 2, :]
                            n_comb += 1
                            if has_chunks[t]:
                                meanT = combpool.tile([HID, P], bf16,
                                                      tag="meanT")
                                nc.vector.tensor_tensor(
                                    out=meanT[:],
                                    in0=ptile_of(q)[
                                        :, (t - q * 4) * P
                                        : (t - q * 4 + 1) * P
                                    ],
                                    in1=minv_sb[:, t * P : (t + 1) * P],
                                    op=mybir.AluOpType.mult,
                                )
                                nc.tensor.matmul(
                                    out=cps, lhsT=meanT[:], rhs=wlt_sb[:],
                                    start=True, stop=False,
                                )
                                nc.tensor.matmul(
                                    out=cps,
                                    lhsT=hT_sb[:, t * P : (t + 1) * P],
                                    rhs=wrt_sb[:],
                                    start=False, stop=True,
                                )
                            else:
                                nc.tensor.matmul(
                                    out=cps,
                                    lhsT=hT_sb[:, t * P : (t + 1) * P],
                                    rhs=wrt_sb[:],
                                    start=True, stop=True,
                                )
                            nc.vector.tensor_tensor(
                                out=out_sb[:, si, :], in0=cps,
                                in1=blb_sb[:],
                                op=mybir.AluOpType.add,
                            )
                        nc.sync.dma_start(
                            out=out_d.ap()[
                                :, q * 4 * HID : q * 4 * HID + len(tset) * HID
                            ],
                            in_=out_sb[:, : len(tset), :],
                        )

                    for ii in range(n_local_instrs, len(instrs)):
                        c0, nch_i, bank = instrs[ii]
                        msgbf, btile = gather_and_btile(c0, nch_i, bank)
                        done_quads = []
                        for k in range(nch_i):
                            ci = c0 + k
                            t = int(sched_t[ci])
                            q = t // 4
                            if ci == rfirst[q]:
                                if q in partials:
                                    emit_idadd(q, stop=False)
                                    st = False
                                else:
                                    st = True
                            else:
                                st = False
                            r = t - q * 4
                            nc.tensor.matmul(
                                out=ptile_of(q)[:, r * P : (r + 1) * P],
                                lhsT=msgbf[:, k, :],
                                rhs=btile[:, k, :],
                                start=st,
                                stop=(ci == rlast[q]),
                            )
                            if ci == rlast[q]:
                                done_quads.append(q)
                        for q in done_quads:
                            combine_quad(q)
                    # quads never touched by remote chunks (local-only)
                    for q in range(N_QUADS):
                        if rfirst[q] < 0:
                            combine_quad(q)

    nc.compile()
    return nc


def kernel(x, edge_index, W1, b1, Wl, bl, Wr):
    from concourse.bass_utils import run_bass_kernel_spmd

    x = np.asarray(x)
    edge_index = np.asarray(edge_index)
    W1 = np.asarray(W1, dtype=np.float32)
    b1v = np.asarray(b1, dtype=np.float32)
    Wl = np.asarray(Wl, dtype=np.float32)
    blv = np.asarray(bl, dtype=np.float32)
    Wr = np.asarray(Wr, dtype=np.float32)

    meta, core_arrays = _prep(edge_index)
    nc = _build_program(meta)

    # host-side transpose of x into PE-ready [P, 4, gw] groups, per core
    w1t_np = np.ascontiguousarray(W1.T).astype(ml_dtypes.bfloat16)
    b1_np = np.ascontiguousarray(b1v[:, None])
    wlt_np = np.ascontiguousarray(Wl.T).astype(ml_dtypes.bfloat16)
    wrt_np = np.ascontiguousarray(Wr.T).astype(ml_dtypes.bfloat16)
    blb_np = np.broadcast_to(blv[None, :], (P, HID)).copy()
    ident_np = np.eye(HID, dtype=ml_dtypes.bfloat16)
    iota_np = np.ascontiguousarray(
        np.broadcast_to(np.arange(P, dtype=np.float32)[None, :], (P, P))
    )

    in_maps = []
    for c in range(N_CORES):
        xc = np.zeros((SHP, XD), dtype=ml_dtypes.bfloat16)
        xc[:SH] = x[c * SH : (c + 1) * SH].astype(ml_dtypes.bfloat16)
        # xg[p, 4*g0 + k*gw + j] = x[g0 + j, 128k + p]
        parts = []
        for g0, gw in GROUPS:
            blk = xc[g0 : g0 + gw].reshape(gw, 4, P).transpose(2, 1, 0)
            parts.append(np.ascontiguousarray(blk).reshape(P, 4 * gw))
        xg_np = np.ascontiguousarray(np.concatenate(parts, axis=1))
        ca = core_arrays[c]
        in_maps.append({
            "xg": xg_np,
            "w1t": w1t_np,
            "b1": b1_np,
            "wlt": wlt_np,
            "wrt": wrt_np,
            "blb": blb_np,
            "ident": ident_np,
            "gidx": ca["gidx"],
            "dstloc": ca["dstloc"],
            "iota": iota_np,
            "minv": ca["minv"],
        })

    global LAST_EXEC_NS, LAST_RES
    res = run_bass_kernel_spmd(nc, in_maps, list(range(N_CORES)), trace=TRACE)
    LAST_EXEC_NS = res.exec_time_ns
    LAST_RES = res
    out = np.empty((N_NODES, HID), dtype=np.float32)
    for c in range(N_CORES):
        # out_d is partition-major: [r, 4i+s tiles x 64]; un-permute to rows
        ob = res.results[c]["out"].astype(np.float32)  # [128, NSTORE*256]
        full = ob.reshape(P, -1, HID).transpose(1, 0, 2).reshape(-1, HID)
        out[c * SH : (c + 1) * SH] = full[:SH]
    return out


# revision 27
# speedup vs baseline: 1.0348x; 1.0348x over previous
"""GNN encoder (Linear+ReLU -> mean-aggregation SAGEConv) on 8 TRN2 NeuronCores.

Self-contained: hardcodes problem shapes (N=100000, XD=512, HID=64, E=1e6).

Strategy (v3):
  - Nodes sharded across 8 cores (12500 each, padded to 12544 = 98 tiles).
  - Phase 1 per core: hT = relu(W1 @ xT + b1) via PE; x fed host-pretransposed
    in PE-ready [128, 4, 512] group layout. hT kept in SBUF (bf16) for the
    combine's root term.
  - h rows are PE-transposed, cast to fp8e4m3 (64B payload in a 256B-stride
    row, required by the SWDGE gather), and stored to ag_in with a global
    partition-major swizzle (node i -> row (i%128)*98 + i//128, so each
    group store is one 128-descriptor DMA).
  - One AllGather -> full 100352-row fp8 table (4 int16-addressable banks of
    25088 rows). A local pseudo-bank was tried and removed: the AllGather's
    own transfers occupy the DMA engines during that window, so overlapping
    local gathers is zero-sum (total DMA work is conserved). Splitting the
    AllGather (per-bank or per-half) also loses: each collective pays ~25us
    of inter-core sync and the later halves contend with phase-2 gathers.
  - Edges partitioned by destination node; per core grouped by (dst tile,
    src bank) in (28-tile block, bank, tile) order; chunks of 128 with a
    shared max-over-cores schedule (SPMD: one program for all 8 cores);
    instrs batch <=6 same-bank chunks (SWDGE ring holds 1024 descriptors;
    6-chunk instrs measured fastest).
  - dma_gather (4 SWDGE rings, ~25-32ns/row: HBM random-access bound)
    fetches fp8 h[src] rows; GpSimd issue time is backpressure-dominated.
  - One-hot B matrices generated ON DEVICE: one DVE tensor_tensor is_equal
    per gather instr (iota row vs per-chunk dstloc, stride-0 broadcast APs),
    fp8 0/1 output. This replaces the ~20MB host-precomputed B stream the
    previous version fetched from HBM.
  - Per chunk: PE matmul lhsT=msg[128,64] fp8 x rhs=B[128,128] accumulates
    sums into per-quad PSUM [64,512] (whole-quad start/stop flags since
    start zeroes the full bank; 7 rotating tiles per 28-tile block).
  - Combine per quad: meanT = sums * minv (full 1/deg in bf16), then
    cps = meanT.T @ WlT + hT.T @ WrT + bl; batched stores; bf16 out, host
    upcasts.  Rel err ~0.0174 (fp8 message noise; gate is 2e-2).
"""

import numpy as np
import ml_dtypes

N_NODES = 100000
XD = 512
HID = 64
N_CORES = 8
SH = N_NODES // N_CORES          # 12500
P = 128
T_TILES = 98                     # ceil(12500/128)
SHP = T_TILES * P                # 12544
NTAB = SHP * N_CORES             # 100352
N_BANKS = 4
BANK = NTAB // N_BANKS           # 25088 (int16-addressable)
BLOCK_TILES = 28                 # tiles per psum block (7 quads)
MAX_CHUNKS_PER_INSTR = 6         # 1024 descriptors (runtime SWDGE ring cap)
SCRATCH = 16384
N_QUADS = (T_TILES + 3) // 4     # 25
GROUPS = [(g * 512, min(512, SHP - g * 512)) for g in range((SHP + 511) // 512)]
# ag_in row order is swizzled so the single phase-1 store is partition-major:
# local node i -> row (i%128)*T_TILES + i//128
_i = np.arange(SHP)
ROW_SWIZ = (_i % P) * T_TILES + _i // P

BLOCKS = [list(range(b0, min(b0 + BLOCK_TILES, T_TILES)))
          for b0 in range(0, T_TILES, BLOCK_TILES)]

TRACE = False          # set True (e.g. from test.py) to profile
LAST_EXEC_NS = None    # filled when TRACE
LAST_RES = None


def _prep(edge_index):
    """Host-side sharding/scheduling. Returns shared schedule + per-core arrays.

    Groups: per dst tile, a LOCAL group (src in own shard, gathered from ltab
    before the AllGather completes) ordered tile-major first, then remote
    groups (4 table banks) in (block, bank, tile) order. Chunks of 128 edges;
    instructions batch <=8 consecutive same-bank chunks.
    """
    src = np.asarray(edge_index[0], dtype=np.int64)
    dst = np.asarray(edge_index[1], dtype=np.int64)
    LB = N_BANKS  # local pseudo-bank

    group_list = [(t, LB) for t in range(T_TILES)]
    for tiles in BLOCKS:
        for b in range(N_BANKS):
            for t in tiles:
                group_list.append((t, b))
    G = len(group_list)
    gid_of = {tb: i for i, tb in enumerate(group_list)}
    gid_lut = np.zeros((T_TILES, N_BANKS + 1), dtype=np.int64)
    for (t, b), i in gid_of.items():
        gid_lut[t, b] = i

    per_core = []
    counts_all = np.zeros((N_CORES, G), dtype=np.int64)
    for c in range(N_CORES):
        sel = (dst >= c * SH) & (dst < (c + 1) * SH)
        e_src = src[sel]
        e_ld = (dst[sel] - c * SH).astype(np.int64)
        deg = np.bincount(e_ld, minlength=SHP)
        minv = (1.0 / np.maximum(deg, 1)).astype(np.float32)
        # local bank disabled: the AG window is occupied by collective traffic
        tid = (e_src // SH) * SHP + ROW_SWIZ[e_src % SH]
        bank = tid // BANK
        blocal = (tid % BANK).astype(np.int64)
        tt = e_ld // P
        gid = gid_lut[tt, bank]
        order = np.argsort(gid * (BANK + 1) + blocal, kind="stable")
        per_core.append({
            "blocal": blocal[order].astype(np.int16),
            "dstloc": (e_ld[order] % P).astype(np.float32),
            "minv_row": minv,
        })
        counts_all[c] = np.bincount(gid, minlength=G)

    q_g = -(-counts_all.max(axis=0) // P)   # chunks per group (shared)

    sched_t = []
    sched_b = []
    for gi, (t, b) in enumerate(group_list):
        for _ in range(q_g[gi]):
            sched_t.append(t)
            sched_b.append(b)
    sched_t = np.array(sched_t, dtype=np.int64)
    sched_b = np.array(sched_b, dtype=np.int64)
    nch = len(sched_t)
    n_local = int((sched_b == LB).sum())

    # instruction list: batch consecutive same-bank chunks (within block for
    # remote; local chunks are all one pseudo-bank)
    instrs = []
    i = 0
    while i < nch:
        j = i
        while (j < nch and j - i < MAX_CHUNKS_PER_INSTR
               and sched_b[j] == sched_b[i]
               and (j < n_local) == (i < n_local)
               and (i < n_local
                    or sched_t[j] // BLOCK_TILES == sched_t[i] // BLOCK_TILES)):
            j += 1
        instrs.append((i, j - i, int(sched_b[i])))
        i = j

    lfirst = np.full(N_QUADS, -1, dtype=np.int64)
    llast = np.full(N_QUADS, -1, dtype=np.int64)
    rfirst = np.full(N_QUADS, -1, dtype=np.int64)
    rlast = np.full(N_QUADS, -1, dtype=np.int64)
    for ci in range(nch):
        q = sched_t[ci] // 4
        if ci < n_local:
            if lfirst[q] < 0:
                lfirst[q] = ci
            llast[q] = ci
        else:
            if rfirst[q] < 0:
                rfirst[q] = ci
            rlast[q] = ci

    # chunk slot offset within its group
    grp_seen = {}
    chunk_q = np.zeros(nch, dtype=np.int64)
    for ci in range(nch):
        k = (int(sched_t[ci]), int(sched_b[ci]))
        chunk_q[ci] = grp_seen.get(k, 0)
        grp_seen[k] = chunk_q[ci] + 1

    core_arrays = []
    for c in range(N_CORES):
        pc = per_core[c]
        cnts = counts_all[c]
        starts = np.zeros(G + 1, dtype=np.int64)
        np.cumsum(cnts, out=starts[1:])
        gidx = np.zeros((nch, P), dtype=np.int16)
        dstloc = np.full((nch, P), 255.0, dtype=np.float32)
        for ci in range(nch):
            t, b, qq = int(sched_t[ci]), int(sched_b[ci]), int(chunk_q[ci])
            g = gid_of[(t, b)]
            s0 = starts[g] + qq * P
            n = min(P, starts[g + 1] - s0)
            if n <= 0:
                continue
            sl = slice(s0, s0 + n)
            gidx[ci, :n] = pc["blocal"][sl]
            dstloc[ci, :n] = pc["dstloc"][sl]
        idx16 = gidx.reshape(nch, 8, 16).transpose(2, 0, 1).reshape(16, nch * 8)
        idx128 = np.tile(idx16, (8, 1))
        core_arrays.append({
            "gidx": np.ascontiguousarray(idx128),
            "dstloc": np.ascontiguousarray(dstloc.T),   # [128, nch]
            "minv": np.ascontiguousarray(
                np.broadcast_to(pc["minv_row"][None, :], (HID, SHP))
            ).astype(ml_dtypes.bfloat16),
        })

    # DoubleRow pairing: per instr, (k, n_k) runs of 1-2 same-tile chunks
    pair_runs = []
    for (c0, nch_i, b) in instrs:
        runs = []
        k = 0
        while k < nch_i:
            if (k + 1 < nch_i and sched_t[c0 + k] == sched_t[c0 + k + 1]):
                runs.append((k, 2))
                k += 2
            else:
                runs.append((k, 1))
                k += 1
        pair_runs.append(runs)

    meta = {
        "nch": nch,
        "n_local": n_local,
        "pair_runs": pair_runs,
        "instrs": instrs,
        "sched_t": sched_t,
        "lfirst": lfirst, "llast": llast,
        "rfirst": rfirst, "rlast": rlast,
        "has_chunks": np.array([
            counts_all.max(axis=0)[
                [gid_of[(t, b)] for b in range(N_BANKS + 1)]
            ].sum() > 0 for t in range(T_TILES)
        ]),
    }
    return meta, core_arrays


_GATHER_PATCHED = False


def _relax_gather_elem_assert():
    """dma_gather asserts elem_size_bytes % 256 == 0 (a transpose-mode
    restriction applied unconditionally). The non-transpose ucode handles
    128-byte payloads with a 256-byte row stride (verified on hardware)."""
    global _GATHER_PATCHED
    if _GATHER_PATCHED:
        return
    import inspect
    import re
    import concourse.bass as bassmod

    src = inspect.getsource(bassmod.BassGpSimd.dma_gather)
    src = src.replace(
        "elem_size_bytes > 0 and elem_size_bytes % 256 == 0",
        "elem_size_bytes > 0 and elem_size_bytes % 64 == 0",
    )
    src = re.sub(r"^    def ", "def ", src, count=1, flags=re.M)
    src = "\n".join(l[4:] if l.startswith("    ") else l for l in src.split("\n"))
    ns = dict(bassmod.__dict__)
    exec(compile(src, "patched_dma_gather", "exec"), ns)
    bassmod.BassGpSimd.dma_gather = ns["dma_gather"]
    _GATHER_PATCHED = True


def _build_program(meta):
    import concourse.bass as bass
    import concourse.bacc as bacc
    import concourse.mybir as mybir
    import concourse.tile as tile

    _relax_gather_elem_assert()

    nch = meta["nch"]
    gcols = nch * 8

    nc = bacc.Bacc("TRN2", target_bir_lowering=False, debug=False,
                   num_devices=N_CORES, num_swdge_queues=4,
                   dynamic_dma_scratch_size=SCRATCH)
    f32 = mybir.dt.float32
    bf16 = mybir.dt.bfloat16

    xg_in = nc.dram_tensor("xg", [P, 4 * SHP], bf16, kind="ExternalInput")
    w1t = nc.dram_tensor("w1t", [XD, HID], bf16, kind="ExternalInput")
    b1 = nc.dram_tensor("b1", [HID, 1], f32, kind="ExternalInput")
    wlt = nc.dram_tensor("wlt", [HID, HID], bf16, kind="ExternalInput")
    wrt = nc.dram_tensor("wrt", [HID, HID], bf16, kind="ExternalInput")
    blb = nc.dram_tensor("blb", [P, HID], f32, kind="ExternalInput")
    ident_in = nc.dram_tensor("ident", [HID, HID], bf16, kind="ExternalInput")
    gidx_in = nc.dram_tensor("gidx", [P, gcols], mybir.dt.int16, kind="ExternalInput")
    dstloc_in = nc.dram_tensor("dstloc", [P, nch], f32, kind="ExternalInput")
    iota_in = nc.dram_tensor("iota", [P, P], f32, kind="ExternalInput")
    minv_in = nc.dram_tensor("minv", [HID, SHP], bf16, kind="ExternalInput")

    NSTORE = sum(-(-len(t) // 4) for t in BLOCKS)
    out_d = nc.dram_tensor("out", [P, NSTORE * 4 * HID], bf16,
                           kind="ExternalOutput")

    fp8 = mybir.dt.float8e4
    ag_in = nc.dram_tensor("ag_in", [SHP, 4 * HID], fp8)
    ag_out = nc.dram_tensor("ag_out", [NTAB, 4 * HID], fp8, addr_space="Shared")

    with tile.TileContext(nc) as tc:
        with (
            tc.tile_pool(name="const", bufs=1) as cpool,
            tc.tile_pool(name="idx", bufs=1) as ipool,
            tc.tile_pool(name="hT", bufs=1) as hpool,
        ):
            w1t_sb = cpool.tile([P, 4, HID], bf16)
            nc.sync.dma_start(
                out=w1t_sb[:],
                in_=w1t.ap().rearrange("(k p) d -> p k d", p=P),
            )
            b1_sb = cpool.tile([HID, 1], f32)
            nc.sync.dma_start(out=b1_sb[:], in_=b1[:])
            wlt_sb = cpool.tile([HID, HID], bf16)
            nc.sync.dma_start(out=wlt_sb[:], in_=wlt[:])
            wrt_sb = cpool.tile([HID, HID], bf16)
            nc.sync.dma_start(out=wrt_sb[:], in_=wrt[:])
            blb_sb = cpool.tile([P, HID], f32)
            nc.sync.dma_start(out=blb_sb[:], in_=blb[:])
            ident_sb = cpool.tile([HID, HID], bf16)
            nc.sync.dma_start(out=ident_sb[:], in_=ident_in[:])
            gidx_sb = ipool.tile([P, gcols], mybir.dt.int16)
            nc.scalar.dma_start(out=gidx_sb[:], in_=gidx_in[:])
            dstloc_sb = ipool.tile([P, nch], f32)
            nc.scalar.dma_start(out=dstloc_sb[:], in_=dstloc_in[:])
            iota_sb = ipool.tile([P, P], f32)
            nc.scalar.dma_start(out=iota_sb[:], in_=iota_in[:])
            minv_sb = ipool.tile([HID, SHP], bf16)
            nc.scalar.dma_start(out=minv_sb[:], in_=minv_in[:])

            hT_sb = hpool.tile([HID, SHP], bf16)
            hrow_all = hpool.tile([P, T_TILES, 4 * HID], fp8)

            # ---------------- Phase 1: hT = relu(W1 @ xT + b1) ----------------
            with (
                tc.tile_pool(name="xg", bufs=6) as xpool,
                tc.tile_pool(name="p1ps", bufs=4, space="PSUM") as p1ps,
                tc.tile_pool(name="p1tr", bufs=4, space="PSUM") as p1tr,
            ):
                for gi, (g0, gw) in enumerate(GROUPS):
                    xt = xpool.tile([P, 4, 512], bf16, tag="xg")
                    xq = nc.scalar
                    xq.dma_start(
                        out=xt[:, :, :gw],
                        in_=xg_in.ap()[:, 4 * g0 : 4 * g0 + 4 * gw].rearrange(
                            "p (k j) -> p k j", k=4
                        ),
                    )
                    hps = p1ps.tile([HID, 512], f32, tag="hps", space="PSUM")
                    for k in range(4):
                        nc.tensor.matmul(
                            out=hps[:, :gw],
                            lhsT=w1t_sb[:, k, :],
                            rhs=xt[:, k, :gw],
                            start=(k == 0),
                            stop=(k == 3),
                        )
                    nc.scalar.activation(
                        out=hT_sb[:, g0 : g0 + gw], in_=hps[:, :gw],
                        func=mybir.ActivationFunctionType.Relu,
                        bias=b1_sb[:], scale=1.0,
                    )
                    ns = gw // P
                    for s in range(ns):
                        tp = p1tr.tile([P, HID], bf16, tag="tp", space="PSUM")
                        nc.tensor.transpose(
                            out=tp[:],
                            in_=hT_sb[:, g0 + s * P : g0 + (s + 1) * P],
                            identity=ident_sb[:],
                        )
                        nc.vector.tensor_copy(
                            out=hrow_all[:, g0 // P + s, :HID], in_=tp[:])
                    nc.sync.dma_start(
                        out=ag_in.ap().rearrange("(p t) d -> p t d", p=P)[
                            :, g0 // P : g0 // P + ns, :],
                        in_=hrow_all[:, g0 // P : g0 // P + ns, :],
                    )

            nc.gpsimd.collective_compute(
                "AllGather",
                mybir.AluOpType.bypass,
                replica_groups=[list(range(N_CORES))],
                ins=[ag_in.ap().opt()],
                outs=[ag_out.ap().opt()],
            )

            # ---------------- Phase 2: gather + aggregate + combine ----------
            LB = N_BANKS
            instrs = meta["instrs"]
            sched_t = meta["sched_t"]
            lfirst, llast = meta["lfirst"], meta["llast"]
            rfirst, rlast = meta["rfirst"], meta["rlast"]
            has_chunks = meta["has_chunks"]
            nch = meta["nch"]

            with (
                tc.tile_pool(name="msgbf", bufs=32) as mbfpool,
                tc.tile_pool(name="bmat", bufs=12) as bpool,
                tc.tile_pool(name="part", bufs=1) as ppool,
                tc.tile_pool(name="cps", bufs=1, space="PSUM") as cpspool,
                tc.tile_pool(name="comb", bufs=6) as combpool,
            ):
                cps_all = cpspool.tile([P, 2, HID], f32, tag="cps", space="PSUM")
                partials = {}
                n_comb = 0
                qn = 0

                def gather_and_btile(c0, nch_i, bank):
                    nonlocal qn
                    ni = nch_i * P
                    msgbf = mbfpool.tile([P, MAX_CHUNKS_PER_INSTR, HID], fp8,
                                         tag="msgbf")
                    src_ap = ag_out[bank * BANK : (bank + 1) * BANK, :HID]
                    nc.gpsimd.dma_gather(
                        msgbf[:, :nch_i, :],
                        src_ap,
                        gidx_sb[:, c0 * 8 : c0 * 8 + nch_i * 8],
                        ni, ni, HID,
                        elem_step=4 * HID,
                        queue_num=qn,
                    )
                    qn = (qn + 1) % 4
                    bt = bpool.tile([P, MAX_CHUNKS_PER_INSTR, P], fp8, tag="bt")
                    nc.vector.tensor_tensor(
                        out=bt[:, :nch_i, :],
                        in0=iota_sb[:].unsqueeze(1).broadcast_to([P, nch_i, P]),
                        in1=dstloc_sb[:, c0 : c0 + nch_i].unsqueeze(2)
                            .broadcast_to([P, nch_i, P]),
                        op=mybir.AluOpType.is_equal,
                    )
                    return msgbf, bt

                # ---- local phase: src in own shard, table = ltab ----
                n_local_instrs = 0
                with tc.tile_pool(name="lq", bufs=2, space="PSUM") as lqpool:
                    lq_tiles = {}
                    for ii, (c0, nch_i, bank) in enumerate(instrs):
                        if bank != LB:
                            break
                        n_local_instrs += 1
                        msgbf, btile = gather_and_btile(c0, nch_i, bank)
                        for k in range(nch_i):
                            ci = c0 + k
                            t = int(sched_t[ci])
                            q = t // 4
                            if q not in lq_tiles:
                                lq_tiles[q] = lqpool.tile(
                                    [HID, 512], f32, tag=f"lq{q % 2}",
                                    name=f"lq_{q}", space="PSUM"
                                )
                            lq = lq_tiles[q]
                            r = t - q * 4
                            nc.tensor.matmul(
                                out=lq[:, r * P : (r + 1) * P],
                                lhsT=msgbf[:, k, :],
                                rhs=btile[:, k, :],
                                start=(ci == lfirst[q]),
                                stop=(ci == llast[q]),
                            )
                            if ci == llast[q]:
                                par = ppool.tile([HID, 512], bf16,
                                                 tag=f"par{q}", name=f"par_{q}")
                                nc.vector.tensor_copy(out=par[:], in_=lq[:])
                                partials[q] = par

                # ---- remote phase ----
                with tc.tile_pool(name="agg", bufs=1, space="PSUM") as apool:
                    ptiles = {}

                    def ptile_of(q):
                        key = q % 7
                        if key not in ptiles or ptiles[key][1] != q:
                            ptiles[key] = (
                                apool.tile(
                                    [HID, 512], f32, tag=f"agg{key}",
                                    name=f"agg_{q}", space="PSUM"
                                ),
                                q,
                            )
                        return ptiles[key][0]

                    def emit_idadd(q, stop):
                        nc.tensor.matmul(
                            out=ptile_of(q)[:],
                            lhsT=ident_sb[:],
                            rhs=partials[q][:],
                            start=True, stop=stop,
                        )

                    def combine_quad(q):
                        tset = list(range(q * 4, min(q * 4 + 4, T_TILES)))
                        nonlocal n_comb
                        if rfirst[q] < 0 and q in partials:
                            emit_idadd(q, stop=True)
                        out_sb = combpool.tile([P, 4, HID], bf16, tag="outsb")
                        for si, t in enumerate(tset):
                            cps = cps_all[:, n_comb % 2, :]
                            n_comb += 1
                            if has_chunks[t]:
                                meanT = combpool.tile([HID, P], bf16,
                                                      tag="meanT")
                                nc.vector.tensor_tensor(
                                    out=meanT[:],
                                    in0=ptile_of(q)[
                                        :, (t - q * 4) * P
                                        : (t - q * 4 + 1) * P
                                    ],
                                    in1=minv_sb[:, t * P : (t + 1) * P],
                                    op=mybir.AluOpType.mult,
                                )
                                nc.tensor.matmul(
                                    out=cps, lhsT=meanT[:], rhs=wlt_sb[:],
                                    start=True, stop=False,
                                )
                                nc.tensor.matmul(
                                    out=cps,
                                    lhsT=hT_sb[:, t * P : (t + 1) * P],
                                    rhs=wrt_sb[:],
                                    start=False, stop=True,
                                )
                            else:
                                nc.tensor.matmul(
                                    out=cps,
                                    lhsT=hT_sb[:, t * P : (t + 1) * P],
                                    rhs=wrt_sb[:],
                                    start=True, stop=True,
                                )
                            nc.vector.tensor_tensor(
                                out=out_sb[:, si, :], in0=cps,
                                in1=blb_sb[:],
                                op=mybir.AluOpType.add,
                            )
                        nc.sync.dma_start(
                            out=out_d.ap()[
                                :, q * 4 * HID : q * 4 * HID + len(tset) * HID
                            ],
                            in_=out_sb[:, : len(tset), :],
                        )

                    for ii in range(n_local_instrs, len(instrs)):
                        c0, nch_i, bank = instrs[ii]
                        msgbf, btile = gather_and_btile(c0, nch_i, bank)
                        done_quads = []
                        for k in range(nch_i):
                            ci = c0 + k
                            t = int(sched_t[ci])
                            q = t // 4
                            if ci == rfirst[q]:
                                if q in partials:
                                    emit_idadd(q, stop=False)
                                    st = False
                                else:
                                    st = True
                            else:
                                st = False
                            r = t - q * 4
                            nc.tensor.matmul(
                                out=ptile_of(q)[:, r * P : (r + 1) * P],
                                lhsT=msgbf[:, k, :],
                                rhs=btile[:, k, :],
                                start=st,
                                stop=(ci == rlast[q]),
                            )
                            if ci == rlast[q]:
                                done_quads.append(q)
                        for q in done_quads:
                            combine_quad(q)
                    # quads never touched by remote chunks (local-only)
                    for q in range(N_QUADS):
                        if rfirst[q] < 0:
                            combine_quad(q)

    nc.compile()
    return nc


def kernel(x, edge_index, W1, b1, Wl, bl, Wr):
    from concourse.bass_utils import run_bass_kernel_spmd

    x = np.asarray(x)
    edge_index = np.asarray(edge_index)
    W1 = np.asarray(W1, dtype=np.float32)
    b1v = np.asarray(b1, dtype=np.float32)
    Wl = np.asarray(Wl, dtype=np.float32)
    blv = np.asarray(bl, dtype=np.float32)
    Wr = np.asarray(Wr, dtype=np.float32)

    meta, core_arrays = _prep(edge_index)
    nc = _build_program(meta)

    # host-side transpose of x into PE-ready [P, 4, gw] groups, per core
    w1t_np = np.ascontiguousarray(W1.T).astype(ml_dtypes.bfloat16)
    b1_np = np.ascontiguousarray(b1v[:, None])
    wlt_np = np.ascontiguousarray(Wl.T).astype(ml_dtypes.bfloat16)
    wrt_np = np.ascontiguousarray(Wr.T).astype(ml_dtypes.bfloat16)
    blb_np = np.broadcast_to(blv[None, :], (P, HID)).copy()
    ident_np = np.eye(HID, dtype=ml_dtypes.bfloat16)
    iota_np = np.ascontiguousarray(
        np.broadcast_to(np.arange(P, dtype=np.float32)[None, :], (P, P))
    )

    in_maps = []
    for c in range(N_CORES):
        xc = np.zeros((SHP, XD), dtype=ml_dtypes.bfloat16)
        xc[:SH] = x[c * SH : (c + 1) * SH].astype(ml_dtypes.bfloat16)
        # xg[p, 4*g0 + k*gw + j] = x[g0 + j, 128k + p]
        parts = []
        for g0, gw in GROUPS:
            blk = xc[g0 : g0 + gw].reshape(gw, 4, P).transpose(2, 1, 0)
            parts.append(np.ascontiguousarray(blk).reshape(P, 4 * gw))
        xg_np = np.ascontiguousarray(np.concatenate(parts, axis=1))
        ca = core_arrays[c]
        in_maps.append({
            "xg": xg_np,
            "w1t": w1t_np,
            "b1": b1_np,
            "wlt": wlt_np,
            "wrt": wrt_np,
            "blb": blb_np,
            "ident": ident_np,
            "gidx": ca["gidx"],
            "dstloc": ca["dstloc"],
            "iota": iota_np,
            "minv": ca["minv"],
        })

    global LAST_EXEC_NS, LAST_RES
    res = run_bass_kernel_spmd(nc, in_maps, list(range(N_CORES)), trace=TRACE)
    LAST_EXEC_NS = res.exec_time_ns
    LAST_RES = res
    out = np.empty((N_NODES, HID), dtype=np.float32)
    for c in range(N_CORES):
        # out_d is partition-major: [r, 4i+s tiles x 64]; un-permute to rows
        ob = res.results[c]["out"].astype(np.float32)  # [128, NSTORE*256]
        full = ob.reshape(P, -1, HID).transpose(1, 0, 2).reshape(-1, HID)
        out[c * SH : (c + 1) * SH] = full[:SH]
    return out


# revision 28
# speedup vs baseline: 1.0509x; 1.0156x over previous
"""GNN encoder (Linear+ReLU -> mean-aggregation SAGEConv) on 8 TRN2 NeuronCores.

Self-contained: hardcodes problem shapes (N=100000, XD=512, HID=64, E=1e6).

Strategy (v3):
  - Nodes sharded across 8 cores (12500 each, padded to 12544 = 98 tiles).
  - Phase 1 per core: hT = relu(W1 @ xT + b1) via PE; x fed host-pretransposed
    in PE-ready [128, 4, 512] group layout. hT kept in SBUF (bf16) for the
    combine's root term.
  - h rows are PE-transposed, cast to fp8e4m3 (64B payload in a 256B-stride
    row, required by the SWDGE gather), and stored to ag_in with a global
    partition-major swizzle (node i -> row (i%128)*98 + i//128, so each
    group store is one 128-descriptor DMA).
  - One AllGather -> full 100352-row fp8 table (4 int16-addressable banks of
    25088 rows). A local pseudo-bank was tried and removed: the AllGather's
    own transfers occupy the DMA engines during that window, so overlapping
    local gathers is zero-sum (total DMA work is conserved). Splitting the
    AllGather (per-bank or per-half) also loses: each collective pays ~25us
    of inter-core sync and the later halves contend with phase-2 gathers.
  - Edges partitioned by destination node; per core grouped by (dst tile,
    src bank) in (28-tile block, bank, tile) order; chunks of 128 with a
    shared max-over-cores schedule (SPMD: one program for all 8 cores);
    instrs batch <=6 same-bank chunks (SWDGE ring holds 1024 descriptors;
    6-chunk instrs measured fastest).
  - dma_gather (4 SWDGE rings, ~25-32ns/row: HBM random-access bound)
    fetches fp8 h[src] rows; GpSimd issue time is backpressure-dominated.
  - One-hot B matrices generated ON DEVICE: one DVE tensor_tensor is_equal
    per gather instr (iota row vs per-chunk dstloc, stride-0 broadcast APs),
    fp8 0/1 output. This replaces the ~20MB host-precomputed B stream the
    previous version fetched from HBM.
  - Per chunk: PE matmul lhsT=msg[128,64] fp8 x rhs=B[128,128] accumulates
    sums into per-quad PSUM [64,512] (whole-quad start/stop flags since
    start zeroes the full bank; 7 rotating tiles per 28-tile block).
  - Combine per quad: meanT = sums * minv (full 1/deg in bf16), then
    cps = meanT.T @ WlT + hT.T @ WrT + bl; batched stores; bf16 out, host
    upcasts.  Rel err ~0.0174 (fp8 message noise; gate is 2e-2).
"""

import numpy as np
import ml_dtypes

N_NODES = 100000
XD = 512
HID = 64
N_CORES = 8
SH = N_NODES // N_CORES          # 12500
P = 128
T_TILES = 98                     # ceil(12500/128)
SHP = T_TILES * P                # 12544
NTAB = SHP * N_CORES             # 100352
N_BANKS = 4
BANK = NTAB // N_BANKS           # 25088 (int16-addressable)
BLOCK_TILES = 28                 # tiles per psum block (7 quads)
MAX_CHUNKS_PER_INSTR = 6         # 1024 descriptors (runtime SWDGE ring cap)
SCRATCH = 16384
N_QUADS = (T_TILES + 3) // 4     # 25
GROUPS = [(g * 512, min(512, SHP - g * 512)) for g in range((SHP + 511) // 512)]
# ag_in row order is swizzled so the single phase-1 store is partition-major:
# local node i -> row (i%128)*T_TILES + i//128
_i = np.arange(SHP)
ROW_SWIZ = (_i % P) * T_TILES + _i // P

BLOCKS = [list(range(b0, min(b0 + BLOCK_TILES, T_TILES)))
          for b0 in range(0, T_TILES, BLOCK_TILES)]

TRACE = False          # set True (e.g. from test.py) to profile
LAST_EXEC_NS = None    # filled when TRACE
LAST_RES = None


def _prep(edge_index):
    """Host-side sharding/scheduling. Returns shared schedule + per-core arrays.

    Groups: per dst tile, a LOCAL group (src in own shard, gathered from ltab
    before the AllGather completes) ordered tile-major first, then remote
    groups (4 table banks) in (block, bank, tile) order. Chunks of 128 edges;
    instructions batch <=8 consecutive same-bank chunks.
    """
    src = np.asarray(edge_index[0], dtype=np.int64)
    dst = np.asarray(edge_index[1], dtype=np.int64)
    LB = N_BANKS  # local pseudo-bank

    group_list = [(t, LB) for t in range(T_TILES)]
    for tiles in BLOCKS:
        for b in range(N_BANKS):
            for t in tiles:
                group_list.append((t, b))
    G = len(group_list)
    gid_of = {tb: i for i, tb in enumerate(group_list)}
    gid_lut = np.zeros((T_TILES, N_BANKS + 1), dtype=np.int64)
    for (t, b), i in gid_of.items():
        gid_lut[t, b] = i

    per_core = []
    counts_all = np.zeros((N_CORES, G), dtype=np.int64)
    for c in range(N_CORES):
        sel = (dst >= c * SH) & (dst < (c + 1) * SH)
        e_src = src[sel]
        e_ld = (dst[sel] - c * SH).astype(np.int64)
        deg = np.bincount(e_ld, minlength=SHP)
        minv = (1.0 / np.maximum(deg, 1)).astype(np.float32)
        # local bank disabled: the AG window is occupied by collective traffic
        tid = (e_src // SH) * SHP + ROW_SWIZ[e_src % SH]
        bank = tid // BANK
        blocal = (tid % BANK).astype(np.int64)
        tt = e_ld // P
        gid = gid_lut[tt, bank]
        order = np.argsort(gid * (BANK + 1) + blocal, kind="stable")
        per_core.append({
            "blocal": blocal[order].astype(np.int16),
            "dstloc": (e_ld[order] % P).astype(np.float32),
            "minv_row": minv,
        })
        counts_all[c] = np.bincount(gid, minlength=G)

    q_g = -(-counts_all.max(axis=0) // P)   # chunks per group (shared)

    sched_t = []
    sched_b = []
    for gi, (t, b) in enumerate(group_list):
        for _ in range(q_g[gi]):
            sched_t.append(t)
            sched_b.append(b)
    sched_t = np.array(sched_t, dtype=np.int64)
    sched_b = np.array(sched_b, dtype=np.int64)
    nch = len(sched_t)
    n_local = int((sched_b == LB).sum())

    # instruction list: batch consecutive same-bank chunks (within block for
    # remote; local chunks are all one pseudo-bank)
    instrs = []
    i = 0
    while i < nch:
        j = i
        while (j < nch and j - i < MAX_CHUNKS_PER_INSTR
               and sched_b[j] == sched_b[i]
               and (j < n_local) == (i < n_local)
               and (i < n_local
                    or sched_t[j] // BLOCK_TILES == sched_t[i] // BLOCK_TILES)):
            j += 1
        instrs.append((i, j - i, int(sched_b[i])))
        i = j

    lfirst = np.full(N_QUADS, -1, dtype=np.int64)
    llast = np.full(N_QUADS, -1, dtype=np.int64)
    rfirst = np.full(N_QUADS, -1, dtype=np.int64)
    rlast = np.full(N_QUADS, -1, dtype=np.int64)
    for ci in range(nch):
        q = sched_t[ci] // 4
        if ci < n_local:
            if lfirst[q] < 0:
                lfirst[q] = ci
            llast[q] = ci
        else:
            if rfirst[q] < 0:
                rfirst[q] = ci
            rlast[q] = ci

    # chunk slot offset within its group
    grp_seen = {}
    chunk_q = np.zeros(nch, dtype=np.int64)
    for ci in range(nch):
        k = (int(sched_t[ci]), int(sched_b[ci]))
        chunk_q[ci] = grp_seen.get(k, 0)
        grp_seen[k] = chunk_q[ci] + 1

    core_arrays = []
    for c in range(N_CORES):
        pc = per_core[c]
        cnts = counts_all[c]
        starts = np.zeros(G + 1, dtype=np.int64)
        np.cumsum(cnts, out=starts[1:])
        gidx = np.zeros((nch, P), dtype=np.int16)
        dstloc = np.full((nch, P), 255.0, dtype=np.float32)
        for ci in range(nch):
            t, b, qq = int(sched_t[ci]), int(sched_b[ci]), int(chunk_q[ci])
            g = gid_of[(t, b)]
            s0 = starts[g] + qq * P
            n = min(P, starts[g + 1] - s0)
            if n <= 0:
                continue
            sl = slice(s0, s0 + n)
            gidx[ci, :n] = pc["blocal"][sl]
            dstloc[ci, :n] = pc["dstloc"][sl]
        idx16 = gidx.reshape(nch, 8, 16).transpose(2, 0, 1).reshape(16, nch * 8)
        idx128 = np.tile(idx16, (8, 1))
        core_arrays.append({
            "gidx": np.ascontiguousarray(idx128),
            "dstloc": np.ascontiguousarray(dstloc.T),   # [128, nch]
            "minv": np.ascontiguousarray(
                np.broadcast_to(pc["minv_row"][None, :], (HID, SHP))
            ).astype(ml_dtypes.bfloat16),
        })

    # DoubleRow pairing: per instr, (k, n_k) runs of 1-2 same-tile chunks
    pair_runs = []
    for (c0, nch_i, b) in instrs:
        runs = []
        k = 0
        while k < nch_i:
            if (k + 1 < nch_i and sched_t[c0 + k] == sched_t[c0 + k + 1]):
                runs.append((k, 2))
                k += 2
            else:
                runs.append((k, 1))
                k += 1
        pair_runs.append(runs)

    meta = {
        "nch": nch,
        "n_local": n_local,
        "pair_runs": pair_runs,
        "instrs": instrs,
        "sched_t": sched_t,
        "lfirst": lfirst, "llast": llast,
        "rfirst": rfirst, "rlast": rlast,
        "has_chunks": np.array([
            counts_all.max(axis=0)[
                [gid_of[(t, b)] for b in range(N_BANKS + 1)]
            ].sum() > 0 for t in range(T_TILES)
        ]),
    }
    return meta, core_arrays


_GATHER_PATCHED = False


def _relax_gather_elem_assert():
    """dma_gather asserts elem_size_bytes % 256 == 0 (a transpose-mode
    restriction applied unconditionally). The non-transpose ucode handles
    128-byte payloads with a 256-byte row stride (verified on hardware)."""
    global _GATHER_PATCHED
    if _GATHER_PATCHED:
        return
    import inspect
    import re
    import concourse.bass as bassmod

    src = inspect.getsource(bassmod.BassGpSimd.dma_gather)
    src = src.replace(
        "elem_size_bytes > 0 and elem_size_bytes % 256 == 0",
        "elem_size_bytes > 0 and elem_size_bytes % 64 == 0",
    )
    src = re.sub(r"^    def ", "def ", src, count=1, flags=re.M)
    src = "\n".join(l[4:] if l.startswith("    ") else l for l in src.split("\n"))
    ns = dict(bassmod.__dict__)
    exec(compile(src, "patched_dma_gather", "exec"), ns)
    bassmod.BassGpSimd.dma_gather = ns["dma_gather"]
    _GATHER_PATCHED = True


def _build_program(meta):
    import concourse.bass as bass
    import concourse.bacc as bacc
    import concourse.mybir as mybir
    import concourse.tile as tile

    _relax_gather_elem_assert()

    nch = meta["nch"]
    gcols = nch * 8

    nc = bacc.Bacc("TRN2", target_bir_lowering=False, debug=False,
                   num_devices=N_CORES, num_swdge_queues=4,
                   dynamic_dma_scratch_size=SCRATCH)
    f32 = mybir.dt.float32
    bf16 = mybir.dt.bfloat16

    xg_in = nc.dram_tensor("xg", [P, 4 * SHP], bf16, kind="ExternalInput")
    w1t = nc.dram_tensor("w1t", [XD, HID], bf16, kind="ExternalInput")
    b1 = nc.dram_tensor("b1", [HID, 1], f32, kind="ExternalInput")
    wlt = nc.dram_tensor("wlt", [HID, HID], bf16, kind="ExternalInput")
    wrt = nc.dram_tensor("wrt", [HID, HID], bf16, kind="ExternalInput")
    blb = nc.dram_tensor("blb", [P, HID], f32, kind="ExternalInput")
    ident_in = nc.dram_tensor("ident", [HID, HID], bf16, kind="ExternalInput")
    gidx_in = nc.dram_tensor("gidx", [P, gcols], mybir.dt.int16, kind="ExternalInput")
    dstloc_in = nc.dram_tensor("dstloc", [P, nch], f32, kind="ExternalInput")
    iota_in = nc.dram_tensor("iota", [P, P], f32, kind="ExternalInput")
    minv_in = nc.dram_tensor("minv", [HID, SHP], bf16, kind="ExternalInput")

    NSTORE = sum(-(-len(t) // 4) for t in BLOCKS)
    out_d = nc.dram_tensor("out", [P, NSTORE * 4 * HID], bf16,
                           kind="ExternalOutput")

    fp8 = mybir.dt.float8e4
    ag_in = nc.dram_tensor("ag_in", [SHP, 4 * HID], fp8)
    ag_out = nc.dram_tensor("ag_out", [NTAB, 4 * HID], fp8, addr_space="Shared")

    with tile.TileContext(nc) as tc:
        with (
            tc.tile_pool(name="const", bufs=1) as cpool,
            tc.tile_pool(name="idx", bufs=1) as ipool,
            tc.tile_pool(name="hT", bufs=1) as hpool,
        ):
            w1t_sb = cpool.tile([P, 4, HID], bf16)
            nc.sync.dma_start(
                out=w1t_sb[:],
                in_=w1t.ap().rearrange("(k p) d -> p k d", p=P),
            )
            b1_sb = cpool.tile([HID, 1], f32)
            nc.sync.dma_start(out=b1_sb[:], in_=b1[:])
            wlt_sb = cpool.tile([HID, HID], bf16)
            nc.sync.dma_start(out=wlt_sb[:], in_=wlt[:])
            wrt_sb = cpool.tile([HID, HID], bf16)
            nc.sync.dma_start(out=wrt_sb[:], in_=wrt[:])
            blb_sb = cpool.tile([P, HID], f32)
            nc.sync.dma_start(out=blb_sb[:], in_=blb[:])
            ident_sb = cpool.tile([HID, HID], bf16)
            nc.sync.dma_start(out=ident_sb[:], in_=ident_in[:])
            gidx_sb = ipool.tile([P, gcols], mybir.dt.int16)
            nc.scalar.dma_start(out=gidx_sb[:], in_=gidx_in[:])
            dstloc_sb = ipool.tile([P, nch], f32)
            nc.scalar.dma_start(out=dstloc_sb[:], in_=dstloc_in[:])
            iota_sb = ipool.tile([P, P], f32)
            nc.scalar.dma_start(out=iota_sb[:], in_=iota_in[:])
            minv_sb = ipool.tile([HID, SHP], bf16)
            nc.scalar.dma_start(out=minv_sb[:], in_=minv_in[:])

            hT_sb = hpool.tile([HID, SHP], bf16)
            hrow_all = hpool.tile([P, T_TILES, 4 * HID], fp8)

            # ---------------- Phase 1: hT = relu(W1 @ xT + b1) ----------------
            with (
                tc.tile_pool(name="xg", bufs=6) as xpool,
                tc.tile_pool(name="p1ps", bufs=4, space="PSUM") as p1ps,
                tc.tile_pool(name="p1tr", bufs=4, space="PSUM") as p1tr,
            ):
                def transpose_and_store(g0, gw):
                    ns = gw // P
                    for s in range(ns):
                        tp = p1tr.tile([P, HID], bf16, tag="tp", space="PSUM")
                        nc.tensor.transpose(
                            out=tp[:],
                            in_=hT_sb[:, g0 + s * P : g0 + (s + 1) * P],
                            identity=ident_sb[:],
                        )
                        nc.vector.tensor_copy(
                            out=hrow_all[:, g0 // P + s, :HID], in_=tp[:])
                    nc.sync.dma_start(
                        out=ag_in.ap().rearrange("(p t) d -> p t d", p=P)[
                            :, g0 // P : g0 // P + ns, :],
                        in_=hrow_all[:, g0 // P : g0 // P + ns, :],
                    )

                prev_group = None
                for gi, (g0, gw) in enumerate(GROUPS):
                    xt = xpool.tile([P, 4, 512], bf16, tag="xg")
                    nc.scalar.dma_start(
                        out=xt[:, :, :gw],
                        in_=xg_in.ap()[:, 4 * g0 : 4 * g0 + 4 * gw].rearrange(
                            "p (k j) -> p k j", k=4
                        ),
                    )
                    hps = p1ps.tile([HID, 512], f32, tag="hps", space="PSUM")
                    for k in range(4):
                        nc.tensor.matmul(
                            out=hps[:, :gw],
                            lhsT=w1t_sb[:, k, :],
                            rhs=xt[:, k, :gw],
                            start=(k == 0),
                            stop=(k == 3),
                        )
                    nc.scalar.activation(
                        out=hT_sb[:, g0 : g0 + gw], in_=hps[:, :gw],
                        func=mybir.ActivationFunctionType.Relu,
                        bias=b1_sb[:], scale=1.0,
                    )
                    # software-pipeline: transposes run one group behind so
                    # the in-order PE never stalls on this group's ReLU
                    if prev_group is not None:
                        transpose_and_store(*prev_group)
                    prev_group = (g0, gw)
                transpose_and_store(*prev_group)

            nc.gpsimd.collective_compute(
                "AllGather",
                mybir.AluOpType.bypass,
                replica_groups=[list(range(N_CORES))],
                ins=[ag_in.ap().opt()],
                outs=[ag_out.ap().opt()],
            )

            # ---------------- Phase 2: gather + aggregate + combine ----------
            LB = N_BANKS
            instrs = meta["instrs"]
            sched_t = meta["sched_t"]
            lfirst, llast = meta["lfirst"], meta["llast"]
            rfirst, rlast = meta["rfirst"], meta["rlast"]
            has_chunks = meta["has_chunks"]
            nch = meta["nch"]

            with (
                tc.tile_pool(name="msgbf", bufs=32) as mbfpool,
                tc.tile_pool(name="bmat", bufs=12) as bpool,
                tc.tile_pool(name="part", bufs=1) as ppool,
                tc.tile_pool(name="cps", bufs=1, space="PSUM") as cpspool,
                tc.tile_pool(name="comb", bufs=6) as combpool,
            ):
                cps_all = cpspool.tile([P, 2, HID], f32, tag="cps", space="PSUM")
                partials = {}
                n_comb = 0
                qn = 0

                def gather_and_btile(c0, nch_i, bank):
                    nonlocal qn
                    ni = nch_i * P
                    msgbf = mbfpool.tile([P, MAX_CHUNKS_PER_INSTR, HID], fp8,
                                         tag="msgbf")
                    src_ap = ag_out[bank * BANK : (bank + 1) * BANK, :HID]
                    nc.gpsimd.dma_gather(
                        msgbf[:, :nch_i, :],
                        src_ap,
                        gidx_sb[:, c0 * 8 : c0 * 8 + nch_i * 8],
                        ni, ni, HID,
                        elem_step=4 * HID,
                        queue_num=qn,
                    )
                    qn = (qn + 1) % 4
                    bt = bpool.tile([P, MAX_CHUNKS_PER_INSTR, P], fp8, tag="bt")
                    nc.vector.tensor_tensor(
                        out=bt[:, :nch_i, :],
                        in0=iota_sb[:].unsqueeze(1).broadcast_to([P, nch_i, P]),
                        in1=dstloc_sb[:, c0 : c0 + nch_i].unsqueeze(2)
                            .broadcast_to([P, nch_i, P]),
                        op=mybir.AluOpType.is_equal,
                    )
                    return msgbf, bt

                # ---- local phase: src in own shard, table = ltab ----
                n_local_instrs = 0
                with tc.tile_pool(name="lq", bufs=2, space="PSUM") as lqpool:
                    lq_tiles = {}
                    for ii, (c0, nch_i, bank) in enumerate(instrs):
                        if bank != LB:
                            break
                        n_local_instrs += 1
                        msgbf, btile = gather_and_btile(c0, nch_i, bank)
                        for k in range(nch_i):
                            ci = c0 + k
                            t = int(sched_t[ci])
                            q = t // 4
                            if q not in lq_tiles:
                                lq_tiles[q] = lqpool.tile(
                                    [HID, 512], f32, tag=f"lq{q % 2}",
                                    name=f"lq_{q}", space="PSUM"
                                )
                            lq = lq_tiles[q]
                            r = t - q * 4
                            nc.tensor.matmul(
                                out=lq[:, r * P : (r + 1) * P],
                                lhsT=msgbf[:, k, :],
                                rhs=btile[:, k, :],
                                start=(ci == lfirst[q]),
                                stop=(ci == llast[q]),
                            )
                            if ci == llast[q]:
                                par = ppool.tile([HID, 512], bf16,
                                                 tag=f"par{q}", name=f"par_{q}")
                                nc.vector.tensor_copy(out=par[:], in_=lq[:])
                                partials[q] = par

                # ---- remote phase ----
                with tc.tile_pool(name="agg", bufs=1, space="PSUM") as apool:
                    ptiles = {}

                    def ptile_of(q):
                        key = q % 7
                        if key not in ptiles or ptiles[key][1] != q:
                            ptiles[key] = (
                                apool.tile(
                                    [HID, 512], f32, tag=f"agg{key}",
                                    name=f"agg_{q}", space="PSUM"
                                ),
                                q,
                            )
                        return ptiles[key][0]

                    def emit_idadd(q, stop):
                        nc.tensor.matmul(
                            out=ptile_of(q)[:],
                            lhsT=ident_sb[:],
                            rhs=partials[q][:],
                            start=True, stop=stop,
                        )

                    def combine_quad(q):
                        tset = list(range(q * 4, min(q * 4 + 4, T_TILES)))
                        nonlocal n_comb
                        if rfirst[q] < 0 and q in partials:
                            emit_idadd(q, stop=True)
                        out_sb = combpool.tile([P, 4, HID], bf16, tag="outsb")
                        for si, t in enumerate(tset):
                            cps = cps_all[:, n_comb % 2, :]
                            n_comb += 1
                            if has_chunks[t]:
                                meanT = combpool.tile([HID, P], bf16,
                                                      tag="meanT")
                                nc.vector.tensor_tensor(
                                    out=meanT[:],
                                    in0=ptile_of(q)[
                                        :, (t - q * 4) * P
                                        : (t - q * 4 + 1) * P
                                    ],
                                    in1=minv_sb[:, t * P : (t + 1) * P],
                                    op=mybir.AluOpType.mult,
                                )
                                nc.tensor.matmul(
                                    out=cps, lhsT=meanT[:], rhs=wlt_sb[:],
                                    start=True, stop=False,
                                )
                                nc.tensor.matmul(
                                    out=cps,
                                    lhsT=hT_sb[:, t * P : (t + 1) * P],
                                    rhs=wrt_sb[:],
                                    start=False, stop=True,
                                )
                            else:
                                nc.tensor.matmul(
                                    out=cps,
                                    lhsT=hT_sb[:, t * P : (t + 1) * P],
                                    rhs=wrt_sb[:],
                                    start=True, stop=True,
                                )
                            nc.vector.tensor_tensor(
                                out=out_sb[:, si, :], in0=cps,
                                in1=blb_sb[:],
                                op=mybir.AluOpType.add,
                            )
                        nc.sync.dma_start(
                            out=out_d.ap()[
                                :, q * 4 * HID : q * 4 * HID + len(tset) * HID
                            ],
                            in_=out_sb[:, : len(tset), :],
                        )

                    for ii in range(n_local_instrs, len(instrs)):
                        c0, nch_i, bank = instrs[ii]
                        msgbf, btile = gather_and_btile(c0, nch_i, bank)
                        done_quads = []
                        for k in range(nch_i):
                            ci = c0 + k
                            t = int(sched_t[ci])
                            q = t // 4
                            if ci == rfirst[q]:
                                if q in partials:
                                    emit_idadd(q, stop=False)
                                    st = False
                                else:
                                    st = True
                            else:
                                st = False
                            r = t - q * 4
                            nc.tensor.matmul(
                                out=ptile_of(q)[:, r * P : (r + 1) * P],
                                lhsT=msgbf[:, k, :],
                                rhs=btile[:, k, :],
                                start=st,
                                stop=(ci == rlast[q]),
                            )
                            if ci == rlast[q]:
                                done_quads.append(q)
                        for q in done_quads:
                            combine_quad(q)
                    # quads never touched by remote chunks (local-only)
                    for q in range(N_QUADS):
                        if rfirst[q] < 0:
                            combine_quad(q)

    nc.compile()
    return nc


def kernel(x, edge_index, W1, b1, Wl, bl, Wr):
    from concourse.bass_utils import run_bass_kernel_spmd

    x = np.asarray(x)
    edge_index = np.asarray(edge_index)
    W1 = np.asarray(W1, dtype=np.float32)
    b1v = np.asarray(b1, dtype=np.float32)
    Wl = np.asarray(Wl, dtype=np.float32)
    blv = np.asarray(bl, dtype=np.float32)
    Wr = np.asarray(Wr, dtype=np.float32)

    meta, core_arrays = _prep(edge_index)
    nc = _build_program(meta)

    # host-side transpose of x into PE-ready [P, 4, gw] groups, per core
    w1t_np = np.ascontiguousarray(W1.T).astype(ml_dtypes.bfloat16)
    b1_np = np.ascontiguousarray(b1v[:, None])
    wlt_np = np.ascontiguousarray(Wl.T).astype(ml_dtypes.bfloat16)
    wrt_np = np.ascontiguousarray(Wr.T).astype(ml_dtypes.bfloat16)
    blb_np = np.broadcast_to(blv[None, :], (P, HID)).copy()
    ident_np = np.eye(HID, dtype=ml_dtypes.bfloat16)
    iota_np = np.ascontiguousarray(
        np.broadcast_to(np.arange(P, dtype=np.float32)[None, :], (P, P))
    )

    in_maps = []
    for c in range(N_CORES):
        xc = np.zeros((SHP, XD), dtype=ml_dtypes.bfloat16)
        xc[:SH] = x[c * SH : (c + 1) * SH].astype(ml_dtypes.bfloat16)
        # xg[p, 4*g0 + k*gw + j] = x[g0 + j, 128k + p]
        parts = []
        for g0, gw in GROUPS:
            blk = xc[g0 : g0 + gw].reshape(gw, 4, P).transpose(2, 1, 0)
            parts.append(np.ascontiguousarray(blk).reshape(P, 4 * gw))
        xg_np = np.ascontiguousarray(np.concatenate(parts, axis=1))
        ca = core_arrays[c]
        in_maps.append({
            "xg": xg_np,
            "w1t": w1t_np,
            "b1": b1_np,
            "wlt": wlt_np,
            "wrt": wrt_np,
            "blb": blb_np,
            "ident": ident_np,
            "gidx": ca["gidx"],
            "dstloc": ca["dstloc"],
            "iota": iota_np,
            "minv": ca["minv"],
        })

    global LAST_EXEC_NS, LAST_RES
    res = run_bass_kernel_spmd(nc, in_maps, list(range(N_CORES)), trace=TRACE)
    LAST_EXEC_NS = res.exec_time_ns
    LAST_RES = res
    out = np.empty((N_NODES, HID), dtype=np.float32)
    for c in range(N_CORES):
        # out_d is partition-major: [r, 4i+s tiles x 64]; un-permute to rows
        ob = res.results[c]["out"].astype(np.float32)  # [128, NSTORE*256]
        full = ob.reshape(P, -1, HID).transpose(1, 0, 2).reshape(-1, HID)
        out[c * SH : (c + 1) * SH] = full[:SH]
    return out
